# revision 53
# baseline (speedup 1.0000x reference)
"""Trainium2 Bass kernel for the DiffKS pipeline:
  x = invert_lpc(y, A_exc)         (order-6 time-varying FIR)
  out = sample_wise_lpc(x, A_loop) (order-2 time-varying all-pole IIR)

Sharding: pure data-parallel over batch B=48 -> 6 rows per core x 8 cores.

v5 design (fp8 coefficient streams + engine rebalance of the v3 kernel):
  * Same math as v3: fp16 even/odd sample planes, K=64 x L=1380 chunks,
    W=4 warmup, seedless Gauss-Seidel (s1_0 = xe) + one odd scan + one
    even scan per slab.
  * A_exc is shipped as fp8 e4m3 (validated: quantizing A_exc alone
    costs 5.4e-3 rel vs the 2e-2 gate; combined with the fp16/GS error
    lands ~1.1e-2).  This halves the dominant HBM stream, giving the
    DMA engines slack so compute is never input-starved.
  * The six even-tap planes are cast fp8->fp16 by the otherwise-idle
    Activation engine so the DVE multiplies stay in the 2x fp16 mode
    (fp8 operands would drop DVE to 1x).  The odd-tap planes feed the
    GpSimd (Pool) tensor_tensor ops as raw fp8 - Pool's cost is
    dtype-independent.  (scalar_tensor_tensor / scans are rejected on
    Pool by the compiler's engine check; plain TensorTensor it is.)
  * Both condensation coefficients are folded on the host in fp32:
    e11 = b1o*b1e + b2o and e10 = b1o*b2e ship as extra b planes
    (b1o, b2e | b1e, e11, e10), so Pool runs nothing but the nine odd
    products and all five b planes leave the congested head window.
  * Split: Pool owns the odd products (pa/pb/pc); DVE owns the even
    FIR, the odd finish, f2, the u-chain and both scans.  For the last
    slab only, Pool's free tail (after its dense mult stream) also takes
    the first odd pair-sum and the f2 partial, trimming the final
    DVE-serial chain.
  * Slab 0 takes its even taps directly as fp16 (no cast latency on the
    critical head path); slabs 1-2 use the fp8+cast path.
  * The global DMA order is hand-scheduled against the cost model's
    delivery timing (transfer pipeline + sem propagation) so both
    engines receive each operand just before its demand time; the final
    even scan of the last slab is split in halves so the first half's
    output DMA overlaps the second half.
"""

import os
import sys

import numpy as np

for _p in ("/opt/trn_rl_repo",):
    if _p not in sys.path:
        sys.path.insert(0, _p)

import ml_dtypes
from concourse import bacc, bass, mybir, tile
from concourse.bass_utils import run_bass_kernel_spmd

B, T = 48, 88200
NCORES = 8
BLOC = B // NCORES        # 6 batch rows per core
K, L = 64, 1380           # chunks x chunk length; K*L = 88320 >= T
L2 = L // 2               # 690 output pairs per chunk
W = 4                     # warmup samples per chunk (even)
W2 = W // 2               # warmup pairs
HP = (W + L) // 2         # 692 pairs per chunk-segment
G = 4                     # leading history slots in the y planes
YW = G + HP

BUFS = int(os.environ.get("KS_BUFS", "3"))
FP8A = int(os.environ.get("KS_FP8A", "1"))     # ship A_exc as fp8 e4m3

MULT = mybir.AluOpType.mult
ADD = mybir.AluOpType.add
F16 = mybir.dt.float16
F8 = mybir.dt.float8e4

_compiled = {}


def _dram_view(handle, offset, dims):
    """Raw strided view of a DRAM tensor: dims = [(stride, count), ...]."""
    return bass.AP(handle, offset, [[s, c] for (s, c) in dims])


def _build_program():
    nc = bacc.Bacc("TRN2", target_bir_lowering=False, debug=False)

    # pre-cut per-chunk windows, fully dense (host replicates warmup overlap)
    y_d = nc.dram_tensor("y_sk", (BLOC, K, 2, YW), F16, kind="ExternalInput")
    # slab 0 even taps in fp16 (planes 0:6 used by slab 0 only)
    ae16_d = nc.dram_tensor("ae16_sk", (2, K, 6, HP), F16, kind="ExternalInput")
    a8_d = nc.dram_tensor("a8_sk", (BLOC, K, 12, HP), F8 if FP8A else F16,
                          kind="ExternalInput")
    b_d = nc.dram_tensor("b_sk", (BLOC, K, 5, HP), F16, kind="ExternalInput")
    out_d = nc.dram_tensor("o_sk", (BLOC, K, 2, L2), F16, kind="ExternalOutput")

    v = nc.vector
    g = nc.gpsimd
    sc = nc.scalar

    slabs = {}

    with tile.TileContext(nc) as tc:
        with tc.tile_pool(name="main", bufs=BUFS) as pool:
            # ---------------- tiles for all three slabs ----------------
            for s in range(3):
                t = {}
                t["yt"] = pool.tile([128, 2, YW], F16, name=f"yt{s}", tag="yt")
                adt = F8 if FP8A else F16
                t["a8e"] = [pool.tile([128, 2, HP], adt, name=f"a8e{s}_{j}", tag=f"a8e{j}")
                            for j in range(3)]
                t["a8o"] = [pool.tile([128, 2, HP], adt, name=f"a8o{s}_{j}", tag=f"a8o{j}")
                            for j in range(3)]
                if s == 0 or FP8A:
                    t["ae"] = [pool.tile([128, 2, HP], F16, name=f"ae{s}_{j}", tag=f"ae{j}")
                               for j in range(3)]
                else:
                    t["ae"] = t["a8e"]
                t["bA"] = pool.tile([128, 2, HP], F16, name=f"bA{s}", tag="bA")
                t["bB"] = pool.tile([128, 3, HP], F16, name=f"bB{s}", tag="bB")
                t["xet"] = pool.tile([128, HP + 1], F16, name=f"xe{s}", tag="xe")
                t["xe"] = t["xet"][:, 1:]
                t["xo"] = pool.tile([128, HP], F16, name=f"xo{s}", tag="xo")
                t["e10"] = pool.tile([128, HP], F16, name=f"e10_{s}", tag="e10")
                t["f2m"] = pool.tile([128, HP], F16, name=f"f2m{s}", tag="f2m")
                t["qa"] = pool.tile([128, 2, HP], F16, name=f"qa{s}", tag="qa")
                t["qb"] = pool.tile([128, 2, HP], F16, name=f"qb{s}", tag="qb")
                t["pa"] = pool.tile([128, 2, HP], F16, name=f"pa{s}", tag="pa")
                t["pb"] = pool.tile([128, 2, HP], F16, name=f"pb{s}", tag="pb")
                t["pc"] = pool.tile([128, 2, HP], F16, name=f"pc{s}", tag="pc")
                t["u1"] = pool.tile([128, HP], F16, name=f"u1_{s}", tag="u1")
                t["u2"] = pool.tile([128, HP], F16, name=f"u2_{s}", tag="u2")
                t["yoe"] = pool.tile([128, HP + 1], F16, name=f"yoe{s}", tag="yoe")
                t["yoo"] = pool.tile([128, HP + 1], F16, name=f"yoo{s}", tag="yoo")
                t["dve_mo_c"] = bool(FP8A) and s == 2
                if t["dve_mo_c"]:
                    t["ao16"] = pool.tile([128, 2, HP], F16, name=f"ao16_{s}", tag="ao16")

                def ypairE(d0, yt=t["yt"]):
                    return yt[:, :, G - d0 : G - d0 + HP]

                def ypairO(d0, yt=t["yt"]):
                    basep = yt[:, :, :]
                    return bass.AP(
                        basep.tensor,
                        basep.offset + (G - d0),
                        [[2 * YW, 128], [YW - 1, 2], [1, HP]],
                    )

                t["ypairE"], t["ypairO"] = ypairE, ypairO
                slabs[s] = t

            # ---------------- DMA emitters ----------------
            def ydma(s):
                nc.sync.dma_start(
                    slabs[s]["yt"][:, :, :],
                    _dram_view(y_d, s * 2 * K * 2 * YW,
                               [(K * 2 * YW, 2), (2 * YW, K), (1, 2 * YW)]),
                )

            def ydma_half(s, c0, c1):
                # columns [c0, c1) of both planes of slab s's y tile
                nc.sync.dma_start(
                    slabs[s]["yt"][:, :, c0:c1],
                    _dram_view(y_d, s * 2 * K * 2 * YW + c0,
                               [(K * 2 * YW, 2), (2 * YW, K), (YW, 2), (1, c1 - c0)]),
                )

            def ae16dma_half(s, lo, c0, c1):
                nc.sync.dma_start(
                    slabs[s]["ae"][lo // 2][:, :, c0:c1],
                    _dram_view(ae16_d, lo * HP + c0,
                               [(K * 6 * HP, 2), (6 * HP, K), (HP, 2), (1, c1 - c0)]),
                )

            def adma(s, lo, hi):
                j = (lo - 6) // 2 if lo >= 6 else lo // 2
                dst = slabs[s]["a8o"][j] if lo >= 6 else slabs[s]["a8e"][j]
                nc.sync.dma_start(
                    dst[:, :, :],
                    _dram_view(a8_d, s * 2 * K * 12 * HP + lo * HP,
                               [(K * 12 * HP, 2), (12 * HP, K), (1, 2 * HP)]),
                )

            def ae16dma(s, lo, hi):
                nc.sync.dma_start(
                    slabs[s]["ae"][lo // 2][:, :, :],
                    _dram_view(ae16_d, lo * HP,
                               [(K * 6 * HP, 2), (6 * HP, K), (1, 2 * HP)]),
                )

            def bdma(s, lo, hi):
                dst = slabs[s]["bA"] if lo == 0 else slabs[s]["bB"]
                n = 2 if lo == 0 else 3
                nc.sync.dma_start(
                    dst[:, :, :],
                    _dram_view(b_d, s * 2 * K * 5 * HP + lo * HP,
                               [(K * 5 * HP, 2), (5 * HP, K), (1, n * HP)]),
                )

            # ---------------- global DMA order ----------------
            # slab0 fp16 evens + fp8 odds interleaved with slab1/2 fp8 even
            # streams (small, feed the Act casts early); b planes placed just
            # ahead of each slab's e10 demand.
            ORD = int(os.environ.get("KS_ORD", "4"))
            head = {
                0: [("y",0),("ao",0,0),("ae",0,0),("ae",0,1),("ao",0,1),("ae",0,2),
                    ("a8",1,0),("bA",0),("y",1),("a8",1,1),("ao",0,2),("a8",1,2)],
                1: [("y",0),("ao",0,0),("ae",0,0),("ae",0,1),("ao",0,1),("a8",1,0),
                    ("ae",0,2),("y",1),("bA",0),("a8",1,1),("ao",0,2),("a8",1,2)],
                2: [("y",0),("ao",0,0),("ae",0,0),("a8",1,0),("ae",0,1),("ao",0,1),
                    ("ae",0,2),("y",1),("bA",0),("a8",1,1),("ao",0,2),("a8",1,2)],
                3: [("y",0),("ae",0,0),("ao",0,0),("ae",0,1),("ao",0,1),("ae",0,2),
                    ("a8",1,0),("bA",0),("y",1),("a8",1,1),("ao",0,2),("a8",1,2)],
                4: [("y",0),("ao",0,0),("ae",0,0),("ae",0,1),("a8",1,0),("ao",0,1),
                    ("ae",0,2),("y",1),("a8",1,1),("ao",0,2),("a8",1,2)],
                5: [("y",0),("ao",0,0),("ae",0,0),("ae",0,1),("a8",1,0),("ao",0,1),
                    ("ae",0,2),("y",1),("a8",1,1),("a8",1,2),("bA",0),("ao",0,2)],
                6: [("y",0),("ao",0,0),("ae",0,0),("a8",1,0),("ae",0,1),("ao",0,1),
                    ("ae",0,2),("y",1),("a8",1,1),("bA",0),("ao",0,2),("a8",1,2)],
                7: [("y",0),("ao",0,0),("ae",0,0),("ae",0,1),("a8",1,0),("ae",0,2),
                    ("ao",0,1),("y",1),("a8",1,1),("bA",0),("ao",0,2),("a8",1,2)],
                8: [("y",0),("ao",0,0),("ae",0,0),("ae",0,1),("a8",1,0),("ao",0,1),
                    ("y",1),("ae",0,2),("a8",1,1),("bA",0),("ao",0,2),("a8",1,2)],
            }[ORD]
            tail = [("a8",2,0),("y",2),("a8",2,1),("a8",2,2),
                    ("ao",1,0),("bA",0),("bB",0),("ao",1,1),("bA",1),("ao",1,2),
                    ("ao",2,0),("ao",2,1),("bB",1),("bA",2),("ao",2,2),("bB",2)]
            for item in head + tail:
                kind = item[0]
                if kind == "y":
                    ydma(item[1])
                elif kind == "ae":
                    ae16dma(item[1], item[2] * 2, item[2] * 2 + 2)
                elif kind == "a8":
                    adma(item[1], item[2] * 2, item[2] * 2 + 2)
                elif kind == "ao":
                    adma(item[1], 6 + item[2] * 2, 8 + item[2] * 2)
                elif kind == "bA":
                    bdma(item[1], 0, 2)
                elif kind == "bB":
                    bdma(item[1], 2, 4)

            # ---------------- guard memsets (Pool) ----------------
            for s in range(3):
                g.memset(slabs[s]["yoo"][:, 0:1], 0.0)
                g.memset(slabs[s]["xet"][:, 0:1], 0.0)

            # ---------------- Act casts ----------------
            if FP8A:
                for s in (1, 2):
                    t = slabs[s]
                    sc.copy(t["ae"][0][:], t["a8e"][0][:])
                    sc.copy(t["ae"][1][:], t["a8e"][1][:])
                    sc.copy(t["ae"][2][:], t["a8e"][2][:])

            # ---------------- per-slab compute ----------------
            def emit_evens(s, c0=0, c1=HP):
                t = slabs[s]
                ae, yt = t["ae"], t["yt"]
                qa, qb, xe = t["qa"], t["qb"], t["xe"]
                ypairE = t["ypairE"]

                def yw(d):
                    # ypairE(d) restricted to columns [c0, c1)
                    return yt[:, :, G - d + c0 : G - d + c1]

                cs = slice(c0, c1)
                v.tensor_mul(qa[:, :, cs], ae[0][:, :, cs], yw(1))
                v.tensor_mul(qb[:, :, cs], ae[1][:, :, cs], yw(2))
                v.tensor_add(qa[:, :, cs], qa[:, :, cs], qb[:, :, cs])
                v.tensor_mul(qb[:, :, cs], ae[2][:, :, cs], yw(3))
                v.tensor_add(qa[:, :, cs], qa[:, :, cs], qb[:, :, cs])
                v.tensor_add(xe[:, cs], qa[:, 0, cs], qa[:, 1, cs])
                v.tensor_add(xe[:, cs], xe[:, cs], yt[:, 0, G + c0 : G + c1])

            def emit_pool(s):
                t = slabs[s]
                t["pool_f2m"] = s == 2
                bA = t["bA"]
                pa, pb, pc = t["pa"], t["pb"], t["pc"]
                ypairO = t["ypairO"]
                g.tensor_mul(pa[:], t["a8o"][0][:], ypairO(0))
                g.tensor_mul(pb[:], t["a8o"][1][:], ypairO(1))
                g.tensor_mul(pc[:], t["a8o"][2][:], ypairO(2))
                if s == 2:
                    g.tensor_add(t["pa"][:], t["pa"][:], t["pb"][:])
                    g.tensor_mul(t["f2m"][:], t["bA"][:, 0, :], t["xe"])

            def emit_chain(s):
                r0 = s * 2
                t = slabs[s]
                xe, xo = t["xe"], t["xo"]
                pa, pb, pc = t["pa"], t["pb"], t["pc"]
                f2m = t["f2m"]
                u1, u2 = t["u1"], t["u2"]
                yoe, yoo = t["yoe"], t["yoo"]
                yt = t["yt"]
                bA, bB = t["bA"], t["bB"]
                b1e, e11 = bB[:, 0, :], bB[:, 1, :]
                e10 = bB[:, 2, :]

                if not t.get("pool_f2m"):
                    v.tensor_mul(f2m[:], bA[:, 0, :], xe)
                    v.tensor_add(pa[:], pa[:], pb[:])
                v.tensor_add(pa[:], pa[:], pc[:])
                v.tensor_add(xo[:], pa[:, 0, :], pa[:, 1, :])
                v.tensor_add(xo[:], xo[:], yt[:, 1, G : G + HP])
                v.tensor_add(xo[:], xo[:], f2m[:])
                xet = t["xet"]
                v.tensor_mul(u2[:], e10[:], xet[:, 0:HP])
                v.tensor_add(u2[:], u2[:], xo[:])
                v.tensor_tensor_scan(yoo[:, 1:], e11, u2[:], 0.0, MULT, ADD)
                odma = nc.sync if s == 2 else nc.scalar
                odma.dma_start(
                    _dram_view(out_d, r0 * K * 2 * L2 + L2,
                               [(K * 2 * L2, 2), (2 * L2, K), (1, L2)]),
                    yoo[:, 1 + W2 : 1 + W2 + L2],
                )
                v.tensor_mul(u1[:], b1e, yoo[:, 0:HP])
                v.tensor_add(u1[:], u1[:], xe)
                if s == 2 and not int(os.environ.get('KS_NOHALF', '0')):
                    # halved final scan: first half's output DMA overlaps the
                    # second half's scan
                    HH = HP // 2
                    C1 = HH - W2  # output cols in [1+W2, 1+HH)
                    v.tensor_tensor_scan(yoe[:, 1 : 1 + HH], bA[:, 1, 0:HH],
                                         u1[:, 0:HH], 0.0, MULT, ADD)
                    nc.sync.dma_start(
                        _dram_view(out_d, r0 * K * 2 * L2,
                                   [(K * 2 * L2, 2), (2 * L2, K), (1, C1)]),
                        yoe[:, 1 + W2 : 1 + HH],
                    )
                    v.tensor_tensor_scan(yoe[:, 1 + HH :], bA[:, 1, HH:],
                                         u1[:, HH:], yoe[:, HH : HH + 1],
                                         MULT, ADD)
                    nc.sync.dma_start(
                        _dram_view(out_d, r0 * K * 2 * L2 + C1,
                                   [(K * 2 * L2, 2), (2 * L2, K), (1, L2 - C1)]),
                        yoe[:, 1 + HH : 1 + W2 + L2],
                    )
                else:
                    v.tensor_tensor_scan(yoe[:, 1:], bA[:, 1, :], u1[:], 0.0, MULT, ADD)
                    nc.scalar.dma_start(
                        _dram_view(out_d, r0 * K * 2 * L2,
                                   [(K * 2 * L2, 2), (2 * L2, K), (1, L2)]),
                        yoe[:, 1 + W2 : 1 + W2 + L2],
                    )

            emit_evens(0)
            emit_pool(0)
            emit_evens(1)
            emit_pool(1)
            emit_evens(2)
            emit_pool(2)
            emit_chain(0)
            emit_chain(1)
            emit_chain(2)

    nc.compile()
    return nc


def _prep_inputs(y, A_exc, A_loop):
    """Dtype conversion, even/odd de-interleave, per-chunk window cut."""
    y = np.asarray(y, dtype=np.float32)
    A_exc = np.asarray(A_exc, dtype=np.float32)
    A_loop = np.asarray(A_loop, dtype=np.float32)
    NE = K * L2
    PRE = 24
    PLEN = PRE + NE + 32
    TE = (T + 1) // 2
    TO = T // 2

    def plane_pair(src, dt=np.float16):
        e = np.zeros((B, PLEN), dt)
        o = np.zeros((B, PLEN), dt)
        e[:, PRE : PRE + TE] = src[:, 0::2]
        o[:, PRE : PRE + TO] = src[:, 1::2]
        return e, o

    def windows(plane, starts, width):
        sw = np.lib.stride_tricks.sliding_window_view(plane, width, axis=1)
        return sw[:, starts, :]

    a_starts = np.arange(K) * L2 + PRE - W2
    y_starts = a_starts - G

    ye, yo = plane_pair(y)
    y_sk = np.empty((B, K, 2, YW), np.float16)
    y_sk[:, :, 0, :] = windows(ye, y_starts, YW)
    y_sk[:, :, 1, :] = windows(yo, y_starts, YW)

    adt = ml_dtypes.float8_e4m3 if FP8A else np.float16
    a_sk = np.empty((B, K, 12, HP), adt)
    ae16_sk = np.empty((2, K, 6, HP), np.float16)
    for k in range(1, 7):
        ae, ao = plane_pair(np.ascontiguousarray(A_exc[:, :, k - 1]), dt=np.float32)
        epl = (k - 1) ^ 1  # [A2e|A1e] [A4e|A3e] [A6e|A5e]
        a_sk[:, :, epl, :] = windows(ae, a_starts, HP).astype(adt)
        a_sk[:, :, 6 + k - 1, :] = windows(ao, a_starts, HP).astype(adt)
        # slab-0 rows per core get fp16 even taps (cast-free head path);
        # rows 0..1 of each core's block -> gathered per core at shard time
        ae16_sk[:, :, epl, :] = 0  # filled per-core below

    # b planes: b1e, b1o, b2e, e11 (e11 folded on host in fp32)
    b1 = -A_loop[:, :, 0]
    b2 = -A_loop[:, :, 1]
    b1e32, b1o32 = plane_pair(b1, dt=np.float32)
    b2e32, b2o32 = plane_pair(b2, dt=np.float32)
    e11_32 = b1o32 * b1e32 + b2o32
    # plane order [b1o, b2e | b1e, e11, e10]; e10/e11 folded on host
    e10_32 = b1o32 * b2e32
    b_sk = np.empty((B, K, 5, HP), np.float16)
    b_sk[:, :, 0, :] = windows(b1o32.astype(np.float16), a_starts, HP)
    b_sk[:, :, 1, :] = windows(b2e32.astype(np.float16), a_starts, HP)
    b_sk[:, :, 2, :] = windows(b1e32.astype(np.float16), a_starts, HP)
    b_sk[:, :, 3, :] = windows(e11_32.astype(np.float16), a_starts, HP)
    b_sk[:, :, 4, :] = windows(e10_32.astype(np.float16), a_starts, HP)

    # fp16 even-tap planes for slab 0 of every core (first 2 rows per core)
    ae_full = np.empty((B, K, 6, HP), np.float16)
    for k in range(1, 7):
        ae, _ = plane_pair(np.ascontiguousarray(A_exc[:, :, k - 1]), dt=np.float32)
        ae_full[:, :, (k - 1) ^ 1, :] = windows(ae.astype(np.float16), a_starts, HP)

    in_maps = []
    for c in range(NCORES):
        r0, r1 = c * BLOC, (c + 1) * BLOC
        in_maps.append(
            {
                "y_sk": np.ascontiguousarray(y_sk[r0:r1]),
                "ae16_sk": np.ascontiguousarray(ae_full[r0 : r0 + 2]),
                "a8_sk": np.ascontiguousarray(a_sk[r0:r1]),
                "b_sk": np.ascontiguousarray(b_sk[r0:r1]),
            }
        )
    return in_maps


def _get_program():
    if "nc" not in _compiled:
        _compiled["nc"] = _build_program()
    return _compiled["nc"]


def run(y, A_exc, A_loop, trace=False, **trace_kwargs):
    """Returns (output, BassKernelResults)."""
    nc = _get_program()
    in_maps = _prep_inputs(y, A_exc, A_loop)
    res = run_bass_kernel_spmd(
        nc, in_maps, list(range(NCORES)), trace=trace, **trace_kwargs
    )
    out = np.empty((B, T), np.float32)
    full = np.empty((BLOC, K, L), np.float32)
    for c in range(NCORES):
        o = res.results[c]["o_sk"]          # (BLOC, K, 2, L2) fp16
        full[:, :, 0::2] = o[:, :, 0, :]
        full[:, :, 1::2] = o[:, :, 1, :]
        out[c * BLOC : (c + 1) * BLOC] = full.reshape(BLOC, K * L)[:, :T]
    return out, res


def kernel(y, A_exc, A_loop):
    out, _ = run(y, A_exc, A_loop)
    return out
